# revision 32
# baseline (speedup 1.0000x reference)
"""MLA-style causal self-attention on 8 Trainium2 NeuronCores.

v7: tensor-parallel over heads (2/core). The c_q latent projection is
REPLICATED per core and pipelined chunk-wise straight into the q-decode
(phase B), so the PE never waits on cross-core traffic; only the small
ckv/k_r latents are sequence-sharded and AllGathered (one collective,
fully hidden under the A+B pipeline). All matmul operands bf16 (full PE
rate at any free size); PSUM accumulation fp32.

Per-core device dataflow (transposed layouts: contraction on partitions):
  A-kv:  kvT[640, 256] = Wkv^T @ xT_local for this core's T-slice
         (ckv 512 | k_r 64 roped locally | pad) -> staged -> AllGather,
         readback into ckvT / krT2 residents mid-phase-B.
  A-cq + B interleaved, per 512-query chunk: c_q^T chunk = Wcq^T @ xT
         chunk (8 groups, PSUM -> bf16 SBUF), then q^T = W_qdec_c^T @
         c_q^T chunk (contraction 1024, SCALE folded host-side), rope
         group first ([qr_h0|qr_h1] packed, block-diag perm matmul).
  C: per (chunk i4, head h): flash-style causal attention, 128-key tiles:
     4 ckv + 1 rope score matmuls -> exp on ACT (bf16) -> stair-mask on
     diagonal tiles -> DVE-accumulated softmax denominator -> 4 y matmuls
     into PSUM. j-loop software-pipelined two tiles deep; diagonal
     512-blocks trimmed at 128-query granularity for i4>0 (the last psy
     matmul is the full-width r=0 tile so accumulation closes cleanly).
     ones-matmul + reciprocal + partition_broadcast -> normalize into
     SBUF-resident y^T (bf16).
  D: out^T = W_out_c^T @ y^T, W_out resident bf16, kc-major for
     stationary reuse, PSUM rotated over 5 tags to dodge WAR stalls at
     mc boundaries -> outT f32 -> HBM.
Host sums the 8 partial outT (row-split TP gather) and transposes.

DMA queue discipline: scalar carries the A/B critical loads (x chunks,
c_q weights, rope tables, W_qdec), gpsimd carries kv weights + the
collective (it blocks on the collective's completion, so everything
after it on gpsimd is C-or-later), sync carries only the AG readbacks
(it heads with the multi-core entry barrier and then blocks on the AG).
"""

import math
from contextlib import ExitStack

import numpy as np
import ml_dtypes

import concourse.bass as bass
import concourse.tile as tile
from concourse import bacc, mybir
from concourse.bass_utils import run_bass_kernel_spmd
from concourse.masks import make_identity

F32 = mybir.dt.float32
F32R = mybir.dt.float32r
BF16 = mybir.dt.bfloat16
AF = mybir.ActivationFunctionType

T_FULL = 2048
E = 2048
KV = 512
QL = 1024
RH = 64
QKH = KV + RH     # 576
NH = 16
NCORES = 8
HPC = NH // NCORES
SCALE = 1.0 / math.sqrt(float(KV))

P = 128
NGQ = QL // P             # 8 c_q groups
NGKV = (KV + 2 * RH) // P  # 5 kv groups: ckv x4 + [kr|pad]
LATF = QL + KV + 2 * RH   # 1664 wlat columns: cq | ckv | kr | pad
TLOC = T_FULL // NCORES   # 256


def _make_rot64(nc, pool):
    """RT0 [64, 64] f32 permutation: RT0[x, y] = 1 iff x == (y+32) % 64, so
    matmul(out, lhsT=RT, rhs=src) gives out[d] = src[(d+32) % 64]."""
    rt0 = pool.tile([RH, RH], F32, tag="rt0")
    nc.gpsimd.memset(rt0[:], 0.0)
    nc.gpsimd.affine_select(
        out=rt0[:], in_=rt0[:], compare_op=mybir.AluOpType.not_equal,
        fill=1.0, base=-32, channel_multiplier=1, pattern=[[-1, RH]],
    )
    nc.gpsimd.affine_select(
        out=rt0[:], in_=rt0[:], compare_op=mybir.AluOpType.not_equal,
        fill=1.0, base=32, channel_multiplier=1, pattern=[[-1, RH]],
    )
    return rt0


def build_kernel(T=T_FULL):
    assert T == T_FULL
    NT512 = T // 512
    NKT = T // P
    EK = E // P            # 16 contraction slices in phase A
    QK = QL // P           # 8 contraction slices in phase B
    DK = HPC * KV // P     # 8 contraction slices in phase D

    nc = bacc.Bacc("TRN2", target_bir_lowering=False, debug=False,
                   num_devices=NCORES)

    xTf = nc.dram_tensor("xTf", [E, T], BF16, kind="ExternalInput").ap()
    xTl = nc.dram_tensor("xTl", [E, TLOC], BF16, kind="ExternalInput").ap()
    wlat = nc.dram_tensor("wlat", [E, LATF], BF16, kind="ExternalInput").ap()
    wqd = nc.dram_tensor("wqd", [QL, HPC * QKH], BF16, kind="ExternalInput").ap()
    wout = nc.dram_tensor("wout", [HPC * KV, E], BF16, kind="ExternalInput").ap()
    cos2d = nc.dram_tensor("cos2", [P, T], BF16, kind="ExternalInput").ap()
    ssin2d = nc.dram_tensor("ssin2", [P, T], BF16, kind="ExternalInput").ap()
    klcosd = nc.dram_tensor("klcos", [RH, TLOC], BF16, kind="ExternalInput").ap()
    klsind = nc.dram_tensor("klsin", [RH, TLOC], BF16, kind="ExternalInput").ap()
    outT = nc.dram_tensor("outT", [E, T], F32, kind="ExternalOutput").ap()

    xf_r = xTf.rearrange("(ko p) t -> p ko t", p=P)
    wl_r = wlat.rearrange("(ko p) m -> p ko m", p=P)

    with tile.TileContext(nc) as tc, ExitStack() as ctx:
        dram = ctx.enter_context(tc.tile_pool(name="dram", bufs=1, space="DRAM"))
        cst = ctx.enter_context(tc.tile_pool(name="cst", bufs=1))
        pp = ctx.enter_context(tc.tile_pool(name="pp", bufs=3, space="PSUM"))
        bw = ctx.enter_context(tc.tile_pool(name="bw", bufs=1))
        kvp1 = ctx.enter_context(tc.tile_pool(name="kvp1", bufs=1))
        cqp = ctx.enter_context(tc.tile_pool(name="cqp", bufs=2))
        actx = ExitStack()
        aw = actx.enter_context(tc.tile_pool(name="aw", bufs=1))

        # ---- critical-path DMAs first (scalar/gpsimd; sync's head is the
        # multi-core entry barrier) ----
        xloc = aw.tile([P, EK, TLOC], BF16, tag="xloc")
        nc.gpsimd.dma_start(xloc[:], xTl.rearrange("(ko p) t -> p ko t", p=P))
        wkv = []
        for g in range(NGKV):
            wg = aw.tile([P, EK, P], BF16, tag=f"wkv{g}", name=f"wkv{g}")
            nc.gpsimd.dma_start(
                wg[:], wl_r[:, :, (NGQ + g) * P : (NGQ + g + 1) * P]
            )
            wkv.append(wg)
        xch = [aw.tile([P, EK, 512], BF16, tag=f"xch{tc_ % 2}",
                       name=f"xch{tc_}") for tc_ in range(2)]
        nc.scalar.dma_start(xch[0][:], xf_r[:, :, 0:512])
        wcq = []
        for g in range(NGQ):
            wg = aw.tile([P, EK, P], BF16, tag=f"wcq{g}", name=f"wcq{g}")
            nc.scalar.dma_start(wg[:], wl_r[:, :, g * P : (g + 1) * P])
            wcq.append(wg)
        klcos = cst.tile([RH, TLOC], BF16, tag="klcos")
        nc.gpsimd.dma_start(klcos[:], klcosd[:])
        klsin = cst.tile([RH, TLOC], BF16, tag="klsin")
        nc.gpsimd.dma_start(klsin[:], klsind[:])
        cos2 = cst.tile([P, T], BF16, tag="cos2")
        nc.scalar.dma_start(cos2[:], cos2d[:])
        ssin2 = cst.tile([P, T], BF16, tag="ssin2")
        nc.scalar.dma_start(ssin2[:], ssin2d[:])
        wqd_sb = bw.tile([P, QK, HPC * QKH], BF16, tag="wqd")
        nc.scalar.dma_start(
            wqd_sb[:], wqd.rearrange("(ko p) m -> p ko m", p=P)
        )

        # rot needed from A-kv on; built on gpsimd early
        rt0 = _make_rot64(nc, cst)
        rot = cst.tile([P, P], BF16, tag="rot")   # block-diag(rot64, rot64)
        nc.gpsimd.memset(rot[:], 0.0)
        nc.gpsimd.dma_start(rot[0:RH, 0:RH], rt0[:])
        nc.gpsimd.dma_start(rot[RH:P, RH:P], rt0[:])

        stage2 = dram.tile([NGKV * P, TLOC], BF16)
        ag2out = dram.tile([NCORES * NGKV * P, TLOC], BF16, addr_space="Shared")
        ag2_r = ag2out[:].rearrange("(r g p) s -> p g r s", p=P, g=NGKV)

        # persistent residents needed from phase B on
        ckvT = kvp1.tile([P, KV // P, T], BF16, tag="ckvT")
        # rows 0:64 and 64:128 both hold roped k_r so each head's score
        # matmul sees lhsT at the same base partition as its rhs qr slice
        krT2 = kvp1.tile([P, T], BF16, tag="krT2")
        qnT = [kvp1.tile([P, KV // P, T], BF16, tag=f"qnT{h}", name=f"qnT{h}")
               for h in range(HPC)]
        qrT2 = kvp1.tile([P, T], BF16, tag="qrT2")  # rows 0:64 h0, 64:128 h1

        # ============ Phase A-kv: local slice -> AllGather ==================
        with tc.tile_pool(name="ast", bufs=3) as ast:
            for g in range(NGKV):
                ps = pp.tile([P, TLOC], F32, tag="mm", name="psA")
                for kc in range(EK):
                    nc.tensor.matmul(
                        ps[:], wkv[g][:, kc, :], xloc[:, kc, :],
                        start=(kc == 0), stop=(kc == EK - 1),
                    )
                st = ast.tile([P, TLOC], BF16, tag="ast", name="ast")
                if g == NGKV - 1:
                    # rows 0:64 = k_r -> rope locally before staging
                    nc.gpsimd.memset(st[RH:P, :], 0.0)
                    kraw = ast.tile([RH, TLOC], BF16, tag="kraw", name="kraw")
                    nc.vector.tensor_copy(kraw[:], ps[0:RH, :])
                    pr = pp.tile([RH, TLOC], F32, tag="mm", name="prk")
                    nc.tensor.matmul(pr[:], rot[0:RH, 0:RH], kraw[:],
                                     start=True, stop=True)
                    nc.vector.tensor_mul(st[0:RH, :], kraw[:], klcos[:])
                    rotk = ast.tile([RH, TLOC], BF16, tag="rotk", name="rotk")
                    nc.vector.tensor_mul(rotk[:], pr[:], klsin[:])
                    nc.vector.tensor_add(st[0:RH, :], st[0:RH, :], rotk[:])
                else:
                    nc.vector.tensor_copy(st[:], ps[:])
                nc.scalar.dma_start(stage2[g * P : (g + 1) * P, :], st[:])
        nc.gpsimd.collective_compute(
            "AllGather", mybir.AluOpType.bypass,
            replica_groups=[list(range(NCORES))],
            ins=[stage2.opt()], outs=[ag2out.opt()],
        )
        # readbacks: sync blocks here on the AG; nothing else rides sync
        # until phase D
        for dc in range(KV // P):
            nc.sync.dma_start(
                ckvT[:, dc, :].rearrange("p (r s) -> p r s", r=NCORES),
                ag2_r[:, dc, :, :],
            )
        nc.sync.dma_start(
            krT2[0:RH, :].rearrange("p (r s) -> p r s", r=NCORES),
            ag2_r[0:RH, 4, :, :],
        )
        nc.sync.dma_start(krT2[RH:P, :], krT2[0:RH, :])

        # late constants; gpsimd executes these after the AG trigger
        # unblocks, still long before C needs them
        ident0 = cst.tile([P, P], F32, tag="ident0")
        make_identity(nc, ident0[:])
        ident = cst.tile([P, P], BF16, tag="ident")
        nc.vector.tensor_copy(ident[:], ident0[:])
        ones0 = cst.tile([P, 1], F32, tag="ones0")
        nc.gpsimd.memset(ones0[:], 1.0)
        ones_col = cst.tile([P, 1], F32R, tag="ones")
        nc.vector.tensor_copy(ones_col[:], ones0[:])
        masks = []
        for r in range(4):
            m0 = cst.tile([P, 512], F32, tag="mask0", name=f"mask{r}")
            nc.gpsimd.memset(m0[:], 1.0)
            nc.gpsimd.affine_select(
                out=m0[:], in_=m0[:], compare_op=mybir.AluOpType.is_ge,
                fill=0.0, base=-P * r, channel_multiplier=-1,
                pattern=[[1, 512]],
            )
            mb = cst.tile([P, 512], BF16, tag=f"maskb{r}", name=f"maskb{r}")
            nc.vector.tensor_copy(mb[:], m0[:])
            masks.append(mb)

        # ============ Phases A-cq + B interleaved per 512-chunk =============
        with tc.tile_pool(name="bs", bufs=2) as bs:
            for tcc in range(NT512):
                qsl = slice(tcc * 512, (tcc + 1) * 512)
                if tcc + 1 < NT512:
                    xn = aw.tile([P, EK, 512], BF16, tag=f"xch{(tcc + 1) % 2}",
                                 name=f"xch{tcc + 1}")
                    nc.scalar.dma_start(
                        xn[:], xf_r[:, :, (tcc + 1) * 512 : (tcc + 2) * 512]
                    )
                    xch[(tcc + 1) % 2] = xn
                cqres = cqp.tile([P, QK, 512], BF16, tag="cqres", name="cqres")
                xc = xch[tcc % 2]
                for g in range(NGQ):
                    ps = pp.tile([P, 512], F32, tag="mm", name="psAq")
                    for kc in range(EK):
                        nc.tensor.matmul(
                            ps[:], wcq[g][:, kc, :], xc[:, kc, :],
                            start=(kc == 0), stop=(kc == EK - 1),
                        )
                    if g % 2 == 0:
                        nc.vector.tensor_copy(cqres[:, g, :], ps[:])
                    else:
                        nc.scalar.copy(cqres[:, g, :], ps[:])
                # B: m-groups: 0 = [qr_h0|qr_h1]; 1-4 = qn_h0; 5-8 = qn_h1
                for gm in range(9):
                    ps = pp.tile([P, 512], F32, tag="mm", name="psB")
                    for kc in range(QK):
                        nc.tensor.matmul(
                            ps[:], wqd_sb[:, kc, gm * P : (gm + 1) * P],
                            cqres[:, kc, :],
                            start=(kc == 0), stop=(kc == QK - 1),
                        )
                    if gm == 0:
                        qraw = bs.tile([P, 512], BF16, tag="qraw", name="qraw")
                        nc.vector.tensor_copy(qraw[:], ps[:])
                        prq = pp.tile([P, 512], F32, tag="mm", name="prq")
                        nc.tensor.matmul(prq[:], rot[:], qraw[:],
                                         start=True, stop=True)
                        nc.vector.tensor_mul(qrT2[:, qsl], qraw[:],
                                             cos2[:, qsl])
                        rotq = bs.tile([P, 512], BF16, tag="rotq", name="rotq")
                        nc.vector.tensor_mul(rotq[:], prq[:], ssin2[:, qsl])
                        nc.vector.tensor_add(qrT2[:, qsl], qrT2[:, qsl],
                                             rotq[:])
                    else:
                        h, dc = (gm - 1) // 4, (gm - 1) % 4
                        if gm % 2 == 1:
                            nc.vector.tensor_copy(qnT[h][:, dc, qsl], ps[:])
                        else:
                            nc.scalar.copy(qnT[h][:, dc, qsl], ps[:])
        actx.close()

        # residents only needed from phase C/D (allocated after aw frees);
        # the wout DMA rides gpsimd post-AG, well before D
        kvp2 = ctx.enter_context(tc.tile_pool(name="kvp2", bufs=1))
        v = kvp2.tile([P, NKT, KV], BF16, tag="v")
        yT = kvp2.tile([P, DK, T], BF16, tag="yT")
        accD = kvp2.tile([P, 512], F32R, tag="accD")
        denb = kvp2.tile([P, 512], F32, tag="denb")
        wop = ctx.enter_context(tc.tile_pool(name="wop", bufs=1))
        wout_sb = wop.tile([P, DK, E], BF16, tag="wout")
        nc.gpsimd.dma_start(
            wout_sb[:], wout.rearrange("(ko p) e -> p ko e", p=P)
        )

        # ============ Phase C: attention ====================================
        with ExitStack() as cctx:
            cs = cctx.enter_context(tc.tile_pool(name="cs", bufs=2))

            # v[t, d] via PE transposes of ckvT (ptr scoped: its 2 PSUM banks
            # free before ppy opens)
            with tc.tile_pool(name="ptr", bufs=2, space="PSUM") as ptr:
                for dc in range(KV // P):
                    for tt in range(NKT):
                        pt = ptr.tile([P, P], BF16, tag="tr", name="pt")
                        nc.tensor.transpose(
                            pt[:], ckvT[:, dc, tt * P : (tt + 1) * P], ident[:]
                        )
                        if tt % 2 == 0:
                            nc.vector.tensor_copy(
                                v[:, tt, dc * P : (dc + 1) * P], pt[:]
                            )
                        else:
                            nc.scalar.copy(
                                v[:, tt, dc * P : (dc + 1) * P], pt[:]
                            )

            ppy = cctx.enter_context(tc.tile_pool(name="ppy", bufs=1,
                                                  space="PSUM"))
            pden = cctx.enter_context(tc.tile_pool(name="pden", bufs=1,
                                                   space="PSUM"))

            for i4 in range(NT512):
                for h in range(HPC):
                    # (ksl, off, mask_r): off = query-column offset into the
                    # 512-chunk (free-dim trimming of diagonal blocks)
                    if i4 == 0:
                        tiles = [(slice(r * P, (r + 1) * P), 0, r)
                                 for r in range(4)]
                    else:
                        tiles = [(slice(j * P, (j + 1) * P), 0, None)
                                 for j in range(4 * i4)]
                        base = 4 * i4
                        for r in (3, 2, 1, 0):
                            tiles.append(
                                (slice((base + r) * P, (base + r + 1) * P),
                                 r * P if r else 0, r)
                            )
                    nj = len(tiles)
                    psy = [ppy.tile([P, 512], F32, tag=f"y{dc}",
                                    name=f"psy{dc}")
                           for dc in range(KV // P)]
                    q0 = i4 * 512

                    def scores(idx):
                        ksl, off, _ = tiles[idx]
                        qs = slice(q0 + off, q0 + 512)
                        ps = pp.tile([P, 512], F32, tag="mm",
                                     name=f"psS{idx % 3}")
                        for dc in range(KV // P):
                            nc.tensor.matmul(
                                ps[:, off:512], ckvT[:, dc, ksl],
                                qnT[h][:, dc, qs],
                                start=(dc == 0), stop=False,
                            )
                        nc.tensor.matmul(
                            ps[:, off:512],
                            krT2[h * RH : (h + 1) * RH, ksl],
                            qrT2[h * RH : (h + 1) * RH, qs],
                            start=False, stop=True,
                        )
                        return ps

                    def post(idx, ps):
                        ksl, off, mr = tiles[idx]
                        se = cs.tile([P, 512], BF16, tag="se", bufs=4,
                                     name="se")
                        nc.scalar.activation(se[:, off:512], ps[:, off:512],
                                             AF.Exp)
                        if mr is not None:
                            nc.vector.tensor_mul(
                                se[:, off:512], se[:, off:512],
                                masks[mr][:, off:512],
                            )
                        if idx == 0:
                            nc.vector.tensor_copy(accD[:], se[:])
                        else:
                            nc.vector.tensor_add(
                                accD[:, off:512], accD[:, off:512],
                                se[:, off:512],
                            )
                        first = (idx == 0)
                        last = (idx == nj - 1)
                        for dc in range(KV // P):
                            nc.tensor.matmul(
                                psy[dc][:, off:512],
                                v[:, ksl.start // P, dc * P : (dc + 1) * P],
                                se[:, off:512],
                                start=first, stop=last,
                            )

                    # two-deep software pipeline: scores run two tiles
                    # ahead of the exp-dependent work
                    ring = [scores(0)]
                    if nj > 1:
                        ring.append(scores(1))
                    for idx in range(2, nj):
                        ring.append(scores(idx))
                        post(idx - 2, ring.pop(0))
                    while ring:
                        post(nj - len(ring), ring.pop(0))

                    psden = pden.tile([1, 512], F32, tag="den", name="psden")
                    nc.tensor.matmul(psden[:], ones_col[:], accD[:],
                                     start=True, stop=True)
                    deninv = cs.tile([1, 512], F32, tag="deninv",
                                     name="deninv")
                    nc.vector.reciprocal_approx_fast(out=deninv[:],
                                                     in_=psden[:])
                    nc.gpsimd.partition_broadcast(denb[:], deninv[:])
                    qsl = slice(i4 * 512, (i4 + 1) * 512)
                    for dc in range(KV // P):
                        nc.vector.tensor_mul(
                            yT[:, h * (KV // P) + dc, qsl], psy[dc][:],
                            denb[:],
                        )

        # ============ Phase D: out^T = W_out_c^T @ y^T ======================
        with ExitStack() as dctx:
            dst = dctx.enter_context(tc.tile_pool(name="dst", bufs=3))
            ppd = dctx.enter_context(tc.tile_pool(name="ppd", bufs=1,
                                                  space="PSUM"))
            for mc in range(E // P):
                psD = [ppd.tile([P, 512], F32,
                                tag=f"d{(mc * NT512 + tcc) % 5}",
                                name=f"psD{tcc}")
                       for tcc in range(NT512)]
                for kc in range(DK):
                    for tcc in range(NT512):
                        nc.tensor.matmul(
                            psD[tcc][:],
                            wout_sb[:, kc, mc * P : (mc + 1) * P],
                            yT[:, kc, tcc * 512 : (tcc + 1) * 512],
                            start=(kc == 0), stop=(kc == DK - 1),
                        )
                    if kc == DK - 1:
                        for tcc in range(NT512):
                            ost = dst.tile([P, 512], F32,
                                           tag=f"ost{tcc % 2}",
                                           name="ost")
                            if tcc % 2 == 0:
                                nc.vector.tensor_copy(ost[:], psD[tcc][:])
                            else:
                                nc.scalar.copy(ost[:], psD[tcc][:])
                            (nc.scalar if tcc % 2 else nc.sync).dma_start(
                                outT[mc * P : (mc + 1) * P,
                                     tcc * 512 : (tcc + 1) * 512],
                                ost[:],
                            )

    nc.compile()
    return nc


_NC_CACHE = {}


def _get_nc(T=T_FULL):
    if T not in _NC_CACHE:
        _NC_CACHE[T] = build_kernel(T)
    return _NC_CACHE[T]


def make_in_maps(x, cos, sin, W_qkv, W_qdec, W_out):
    bf = ml_dtypes.bfloat16
    x = np.asarray(x)
    xT = np.ascontiguousarray(x[0].T).astype(bf)           # [E, T]
    W_qkv = np.asarray(W_qkv).astype(np.float32)
    W_qdec = np.asarray(W_qdec).astype(np.float32)
    W_out = np.asarray(W_out).astype(np.float32)
    cos = np.asarray(cos).astype(np.float32)
    sin = np.asarray(sin).astype(np.float32)

    # Wlat columns: cq 1024 | ckv 512 | kr 64 | pad 64  (replicated)
    wlat = np.zeros((E, LATF), np.float32)
    wlat[:, 0:QL] = W_qkv[:, QKH:]
    wlat[:, QL : QL + KV] = W_qkv[:, 0:KV]
    wlat[:, QL + KV : QL + KV + RH] = W_qkv[:, KV : KV + RH]
    wlat = wlat.astype(bf)

    cosT = cos.T.copy()                                     # [64, T]
    ssinT = sin.T.copy()
    ssinT[0 : RH // 2] *= -1.0
    cos2 = np.vstack([cosT, cosT]).astype(bf)               # [128, T]
    ssin2 = np.vstack([ssinT, ssinT]).astype(bf)

    Wq = W_qdec * SCALE
    in_maps = []
    for c in range(NCORES):
        h0, h1 = 2 * c, 2 * c + 1
        # wqd cols: [qr_h0 64 | qr_h1 64 | qn_h0 512 | qn_h1 512]
        wqd_c = np.concatenate(
            [
                Wq[:, h0 * QKH + KV : (h0 + 1) * QKH],
                Wq[:, h1 * QKH + KV : (h1 + 1) * QKH],
                Wq[:, h0 * QKH : h0 * QKH + KV],
                Wq[:, h1 * QKH : h1 * QKH + KV],
            ],
            axis=1,
        ).astype(bf)
        tsl = slice(c * TLOC, (c + 1) * TLOC)
        in_maps.append({
            "xTf": xT,
            "xTl": np.ascontiguousarray(xT[:, tsl]),
            "wlat": wlat,
            "wqd": wqd_c,
            "wout": W_out[c * HPC * KV : (c + 1) * HPC * KV].astype(bf),
            "cos2": cos2,
            "ssin2": ssin2,
            "klcos": np.ascontiguousarray(cos2[0:RH, tsl]),
            "klsin": np.ascontiguousarray(ssin2[0:RH, tsl]),
        })
    return in_maps


def kernel(x, cos, sin, W_qkv, W_qdec, W_out, _trace=False, _tmpdir=None):
    T = np.asarray(x).shape[1]
    nc = _get_nc(T)
    in_maps = make_in_maps(x, cos, sin, W_qkv, W_qdec, W_out)
    res = run_bass_kernel_spmd(
        nc, in_maps, core_ids=list(range(NCORES)),
        trace=_trace, tmpdir=_tmpdir,
    )
    out = np.zeros((E, T), np.float32)
    for r in res.results:
        out += r["outT"]
    kernel.last_results = res
    return np.ascontiguousarray(out.T)[None].astype(np.float32)


# revision 35
# speedup vs baseline: 1.1679x; 1.1679x over previous
"""MLA-style causal self-attention on 8 Trainium2 NeuronCores.

v7: tensor-parallel over heads (2/core). The c_q latent projection is
REPLICATED per core and pipelined chunk-wise straight into the q-decode
(phase B), so the PE never waits on cross-core traffic; only the small
ckv/k_r latents are sequence-sharded and AllGathered (one collective,
fully hidden under the A+B pipeline). All matmul operands bf16 (full PE
rate at any free size); PSUM accumulation fp32.

Per-core device dataflow (transposed layouts: contraction on partitions):
  A-kv:  kvT[640, 256] = Wkv^T @ xT_local for this core's T-slice
         (ckv 512 | k_r 64 roped locally | pad) -> staged -> AllGather,
         readback into ckvT / krT2 residents mid-phase-B.
  A-cq + B interleaved, per 512-query chunk: c_q^T chunk = Wcq^T @ xT
         chunk (8 groups, PSUM -> bf16 SBUF), then q^T = W_qdec_c^T @
         c_q^T chunk (contraction 1024, SCALE folded host-side), rope
         group first ([qr_h0|qr_h1] packed, block-diag perm matmul).
  C: per (chunk i4, head h): flash-style causal attention, 128-key tiles:
     4 ckv + 1 rope score matmuls -> exp on ACT (bf16) -> stair-mask on
     diagonal tiles -> DVE-accumulated softmax denominator -> 4 y matmuls
     into PSUM. j-loop software-pipelined two tiles deep; diagonal
     512-blocks trimmed at 128-query granularity for i4>0 (the last psy
     matmul is the full-width r=0 tile so accumulation closes cleanly).
     ones-matmul + reciprocal + partition_broadcast -> normalize into
     SBUF-resident y^T (bf16).
  D: out^T = W_out_c^T @ y^T, W_out resident bf16, kc-major for
     stationary reuse, PSUM rotated over 5 tags to dodge WAR stalls at
     mc boundaries -> outT f32 -> HBM.
Host sums the 8 partial outT (row-split TP gather) and transposes.

DMA queue discipline: scalar carries the A/B critical loads (x chunks,
c_q weights, rope tables, W_qdec), gpsimd carries kv weights + the
collective (it blocks on the collective's completion, so everything
after it on gpsimd is C-or-later), sync carries only the AG readbacks
(it heads with the multi-core entry barrier and then blocks on the AG).
"""

import math
from contextlib import ExitStack

import numpy as np
import ml_dtypes

import concourse.bass as bass
import concourse.tile as tile
from concourse import bacc, mybir
from concourse.bass_utils import run_bass_kernel_spmd
from concourse.masks import make_identity

F32 = mybir.dt.float32
F32R = mybir.dt.float32r
BF16 = mybir.dt.bfloat16
AF = mybir.ActivationFunctionType

T_FULL = 2048
E = 2048
KV = 512
QL = 1024
RH = 64
QKH = KV + RH     # 576
NH = 16
NCORES = 8
HPC = NH // NCORES
SCALE = 1.0 / math.sqrt(float(KV))

P = 128
NGQ = QL // P             # 8 c_q groups
NGKV = (KV + 2 * RH) // P  # 5 kv groups: ckv x4 + [kr|pad]
LATF = QL + KV + 2 * RH   # 1664 wlat columns: cq | ckv | kr | pad
TLOC = T_FULL // NCORES   # 256


def _make_rot64(nc, pool):
    """RT0 [64, 64] f32 permutation: RT0[x, y] = 1 iff x == (y+32) % 64, so
    matmul(out, lhsT=RT, rhs=src) gives out[d] = src[(d+32) % 64]."""
    rt0 = pool.tile([RH, RH], F32, tag="rt0")
    nc.gpsimd.memset(rt0[:], 0.0)
    nc.gpsimd.affine_select(
        out=rt0[:], in_=rt0[:], compare_op=mybir.AluOpType.not_equal,
        fill=1.0, base=-32, channel_multiplier=1, pattern=[[-1, RH]],
    )
    nc.gpsimd.affine_select(
        out=rt0[:], in_=rt0[:], compare_op=mybir.AluOpType.not_equal,
        fill=1.0, base=32, channel_multiplier=1, pattern=[[-1, RH]],
    )
    return rt0


def build_kernel(T=T_FULL):
    assert T == T_FULL
    NT512 = T // 512
    NKT = T // P
    EK = E // P            # 16 contraction slices in phase A
    QK = QL // P           # 8 contraction slices in phase B
    DK = HPC * KV // P     # 8 contraction slices in phase D

    nc = bacc.Bacc("TRN2", target_bir_lowering=False, debug=False,
                   num_devices=NCORES)

    xTf = nc.dram_tensor("xTf", [E, T], BF16, kind="ExternalInput").ap()
    xTl = nc.dram_tensor("xTl", [E, TLOC], BF16, kind="ExternalInput").ap()
    wlat = nc.dram_tensor("wlat", [E, LATF], BF16, kind="ExternalInput").ap()
    wqd = nc.dram_tensor("wqd", [QL, HPC * QKH], BF16, kind="ExternalInput").ap()
    wout = nc.dram_tensor("wout", [HPC * KV, E], BF16, kind="ExternalInput").ap()
    cos2d = nc.dram_tensor("cos2", [P, T], BF16, kind="ExternalInput").ap()
    ssin2d = nc.dram_tensor("ssin2", [P, T], BF16, kind="ExternalInput").ap()
    klcosd = nc.dram_tensor("klcos", [RH, TLOC], BF16, kind="ExternalInput").ap()
    klsind = nc.dram_tensor("klsin", [RH, TLOC], BF16, kind="ExternalInput").ap()
    outT = nc.dram_tensor("outT", [E, T], F32, kind="ExternalOutput").ap()

    xf_r = xTf.rearrange("(ko p) t -> p ko t", p=P)
    wl_r = wlat.rearrange("(ko p) m -> p ko m", p=P)

    with tile.TileContext(nc) as tc, ExitStack() as ctx:
        dram = ctx.enter_context(tc.tile_pool(name="dram", bufs=1, space="DRAM"))
        cst = ctx.enter_context(tc.tile_pool(name="cst", bufs=1))
        pp = ctx.enter_context(tc.tile_pool(name="pp", bufs=3, space="PSUM"))
        bw = ctx.enter_context(tc.tile_pool(name="bw", bufs=1))
        kvp1 = ctx.enter_context(tc.tile_pool(name="kvp1", bufs=1))
        cqp = ctx.enter_context(tc.tile_pool(name="cqp", bufs=2))
        actx = ExitStack()
        aw = actx.enter_context(tc.tile_pool(name="aw", bufs=1))

        # ---- critical-path DMAs first (scalar/gpsimd; sync's head is the
        # multi-core entry barrier of unpredictable length) ----
        xch = [aw.tile([P, EK, 512], BF16, tag=f"xch{tc_ % 2}",
                       name=f"xch{tc_}") for tc_ in range(2)]
        nc.gpsimd.dma_start(xch[0][:], xf_r[:, :, 0:512])
        klcos = cst.tile([RH, TLOC], BF16, tag="klcos")
        nc.gpsimd.dma_start(klcos[:], klcosd[:])
        klsin = cst.tile([RH, TLOC], BF16, tag="klsin")
        nc.gpsimd.dma_start(klsin[:], klsind[:])
        # rot needed from A-kv on; built on gpsimd early
        rt0 = _make_rot64(nc, cst)
        rot = cst.tile([P, P], BF16, tag="rot")   # block-diag(rot64, rot64)
        nc.gpsimd.memset(rot[:], 0.0)
        nc.gpsimd.dma_start(rot[0:RH, 0:RH], rt0[:])
        nc.gpsimd.dma_start(rot[RH:P, RH:P], rt0[:])
        wcq = []
        for g in range(NGQ):
            wg = aw.tile([P, EK, P], BF16, tag=f"wcq{g}", name=f"wcq{g}")
            nc.gpsimd.dma_start(wg[:], wl_r[:, :, g * P : (g + 1) * P])
            wcq.append(wg)

        xloc = aw.tile([P, EK, TLOC], BF16, tag="xloc")
        nc.scalar.dma_start(xloc[:], xTl.rearrange("(ko p) t -> p ko t", p=P))
        wkv = []
        for g in range(NGKV):
            wg = aw.tile([P, EK, P], BF16, tag=f"wkv{g}", name=f"wkv{g}")
            nc.scalar.dma_start(
                wg[:], wl_r[:, :, (NGQ + g) * P : (NGQ + g + 1) * P]
            )
            wkv.append(wg)
        wqd_sb = bw.tile([P, QK, HPC * QKH], BF16, tag="wqd")
        nc.scalar.dma_start(
            wqd_sb[:], wqd.rearrange("(ko p) m -> p ko m", p=P)
        )
        cos2 = cst.tile([P, T], BF16, tag="cos2")
        nc.scalar.dma_start(cos2[:], cos2d[:])
        ssin2 = cst.tile([P, T], BF16, tag="ssin2")
        nc.scalar.dma_start(ssin2[:], ssin2d[:])

        stage2 = dram.tile([NGKV * P, TLOC], BF16)
        ag2out = dram.tile([NCORES * NGKV * P, TLOC], BF16, addr_space="Shared")
        ag2_r = ag2out[:].rearrange("(r g p) s -> p g r s", p=P, g=NGKV)

        # persistent residents needed from phase B on
        ckvT = kvp1.tile([P, KV // P, T], BF16, tag="ckvT")
        # rows 0:64 and 64:128 both hold roped k_r so each head's score
        # matmul sees lhsT at the same base partition as its rhs qr slice
        krT2 = kvp1.tile([P, T], BF16, tag="krT2")
        qnT = [kvp1.tile([P, KV // P, T], BF16, tag=f"qnT{h}", name=f"qnT{h}")
               for h in range(HPC)]
        qrT2 = kvp1.tile([P, T], BF16, tag="qrT2")  # rows 0:64 h0, 64:128 h1

        # ============ Phase A-kv: local slice -> AllGather ==================
        with tc.tile_pool(name="ast", bufs=3) as ast:
            for g in range(NGKV):
                ps = pp.tile([P, TLOC], F32, tag="mm", name="psA")
                for kc in range(EK):
                    nc.tensor.matmul(
                        ps[:], wkv[g][:, kc, :], xloc[:, kc, :],
                        start=(kc == 0), stop=(kc == EK - 1),
                    )
                st = ast.tile([P, TLOC], BF16, tag="ast", name="ast")
                if g == NGKV - 1:
                    # rows 0:64 = k_r -> rope locally before staging
                    nc.gpsimd.memset(st[RH:P, :], 0.0)
                    kraw = ast.tile([RH, TLOC], BF16, tag="kraw", name="kraw")
                    nc.vector.tensor_copy(kraw[:], ps[0:RH, :])
                    pr = pp.tile([RH, TLOC], F32, tag="mm", name="prk")
                    nc.tensor.matmul(pr[:], rot[0:RH, 0:RH], kraw[:],
                                     start=True, stop=True)
                    nc.vector.tensor_mul(st[0:RH, :], kraw[:], klcos[:])
                    rotk = ast.tile([RH, TLOC], BF16, tag="rotk", name="rotk")
                    nc.vector.tensor_mul(rotk[:], pr[:], klsin[:])
                    nc.vector.tensor_add(st[0:RH, :], st[0:RH, :], rotk[:])
                else:
                    nc.vector.tensor_copy(st[:], ps[:])
                nc.scalar.dma_start(stage2[g * P : (g + 1) * P, :], st[:])
        nc.gpsimd.collective_compute(
            "AllGather", mybir.AluOpType.bypass,
            replica_groups=[list(range(NCORES))],
            ins=[stage2.opt()], outs=[ag2out.opt()],
        )
        # readbacks: sync blocks here on the AG; nothing else rides sync
        # until phase D
        for dc in range(KV // P):
            nc.sync.dma_start(
                ckvT[:, dc, :].rearrange("p (r s) -> p r s", r=NCORES),
                ag2_r[:, dc, :, :],
            )
        nc.sync.dma_start(
            krT2[0:RH, :].rearrange("p (r s) -> p r s", r=NCORES),
            ag2_r[0:RH, 4, :, :],
        )
        nc.sync.dma_start(krT2[RH:P, :], krT2[0:RH, :])

        # ============ Phases A-cq + B interleaved per 512-chunk =============
        with tc.tile_pool(name="bs", bufs=2) as bs:
            for tcc in range(NT512):
                qsl = slice(tcc * 512, (tcc + 1) * 512)
                if tcc + 1 < NT512:
                    xn = aw.tile([P, EK, 512], BF16, tag=f"xch{(tcc + 1) % 2}",
                                 name=f"xch{tcc + 1}")
                    nc.scalar.dma_start(
                        xn[:], xf_r[:, :, (tcc + 1) * 512 : (tcc + 2) * 512]
                    )
                    xch[(tcc + 1) % 2] = xn
                cqres = cqp.tile([P, QK, 512], BF16, tag="cqres", name="cqres")
                xc = xch[tcc % 2]
                for g in range(NGQ):
                    ps = pp.tile([P, 512], F32, tag="mm", name="psAq")
                    for kc in range(EK):
                        nc.tensor.matmul(
                            ps[:], wcq[g][:, kc, :], xc[:, kc, :],
                            start=(kc == 0), stop=(kc == EK - 1),
                        )
                    if g % 2 == 0:
                        nc.vector.tensor_copy(cqres[:, g, :], ps[:])
                    else:
                        nc.scalar.copy(cqres[:, g, :], ps[:])
                # B: m-groups: 0 = [qr_h0|qr_h1]; 1-4 = qn_h0; 5-8 = qn_h1
                for gm in range(9):
                    ps = pp.tile([P, 512], F32, tag="mm", name="psB")
                    for kc in range(QK):
                        nc.tensor.matmul(
                            ps[:], wqd_sb[:, kc, gm * P : (gm + 1) * P],
                            cqres[:, kc, :],
                            start=(kc == 0), stop=(kc == QK - 1),
                        )
                    if gm == 0:
                        qraw = bs.tile([P, 512], BF16, tag="qraw", name="qraw")
                        nc.vector.tensor_copy(qraw[:], ps[:])
                        prq = pp.tile([P, 512], F32, tag="mm", name="prq")
                        nc.tensor.matmul(prq[:], rot[:], qraw[:],
                                         start=True, stop=True)
                        nc.vector.tensor_mul(qrT2[:, qsl], qraw[:],
                                             cos2[:, qsl])
                        rotq = bs.tile([P, 512], BF16, tag="rotq", name="rotq")
                        nc.vector.tensor_mul(rotq[:], prq[:], ssin2[:, qsl])
                        nc.vector.tensor_add(qrT2[:, qsl], qrT2[:, qsl],
                                             rotq[:])
                    else:
                        h, dc = (gm - 1) // 4, (gm - 1) % 4
                        if gm % 2 == 1:
                            nc.vector.tensor_copy(qnT[h][:, dc, qsl], ps[:])
                        else:
                            nc.scalar.copy(qnT[h][:, dc, qsl], ps[:])
        actx.close()

        # late constants: issued after A+B in program order so the mask
        # copies (DVE, gated on gpsimd work that queues behind the blocking
        # AG trigger) can never head-of-line-block phase B's vector queue
        ident0 = cst.tile([P, P], F32, tag="ident0")
        make_identity(nc, ident0[:])
        ident = cst.tile([P, P], BF16, tag="ident")
        nc.vector.tensor_copy(ident[:], ident0[:])
        ones0 = cst.tile([P, 1], F32, tag="ones0")
        nc.gpsimd.memset(ones0[:], 1.0)
        ones_col = cst.tile([P, 1], F32R, tag="ones")
        nc.vector.tensor_copy(ones_col[:], ones0[:])
        masks = []
        for r in range(4):
            m0 = cst.tile([P, 512], F32, tag="mask0", name=f"mask{r}")
            nc.gpsimd.memset(m0[:], 1.0)
            nc.gpsimd.affine_select(
                out=m0[:], in_=m0[:], compare_op=mybir.AluOpType.is_ge,
                fill=0.0, base=-P * r, channel_multiplier=-1,
                pattern=[[1, 512]],
            )
            mb = cst.tile([P, 512], BF16, tag=f"maskb{r}", name=f"maskb{r}")
            nc.vector.tensor_copy(mb[:], m0[:])
            masks.append(mb)

        # residents only needed from phase C/D (allocated after aw frees);
        # the wout DMA rides gpsimd post-AG, well before D
        kvp2 = ctx.enter_context(tc.tile_pool(name="kvp2", bufs=1))
        v = kvp2.tile([P, NKT, KV], BF16, tag="v")
        yT = kvp2.tile([P, DK, T], BF16, tag="yT")
        accD = kvp2.tile([P, 512], F32R, tag="accD")
        denb = kvp2.tile([P, 512], F32, tag="denb")
        wop = ctx.enter_context(tc.tile_pool(name="wop", bufs=1))
        wout_sb = wop.tile([P, DK, E], BF16, tag="wout")
        nc.gpsimd.dma_start(
            wout_sb[:], wout.rearrange("(ko p) e -> p ko e", p=P)
        )

        # ============ Phase C: attention ====================================
        with ExitStack() as cctx:
            cs = cctx.enter_context(tc.tile_pool(name="cs", bufs=2))

            # v[t, d] via PE transposes of ckvT (ptr scoped: its 2 PSUM banks
            # free before ppy opens)
            with tc.tile_pool(name="ptr", bufs=2, space="PSUM") as ptr:
                for dc in range(KV // P):
                    for tt in range(NKT):
                        pt = ptr.tile([P, P], BF16, tag="tr", name="pt")
                        nc.tensor.transpose(
                            pt[:], ckvT[:, dc, tt * P : (tt + 1) * P], ident[:]
                        )
                        if tt % 2 == 0:
                            nc.vector.tensor_copy(
                                v[:, tt, dc * P : (dc + 1) * P], pt[:]
                            )
                        else:
                            nc.scalar.copy(
                                v[:, tt, dc * P : (dc + 1) * P], pt[:]
                            )

            ppy = cctx.enter_context(tc.tile_pool(name="ppy", bufs=1,
                                                  space="PSUM"))
            pden = cctx.enter_context(tc.tile_pool(name="pden", bufs=1,
                                                   space="PSUM"))

            for i4 in range(NT512):
                for h in range(HPC):
                    # (ksl, off, mask_r): off = query-column offset into the
                    # 512-chunk (free-dim trimming of diagonal blocks)
                    if i4 == 0:
                        tiles = [(slice(r * P, (r + 1) * P), 0, r)
                                 for r in range(4)]
                    else:
                        tiles = [(slice(j * P, (j + 1) * P), 0, None)
                                 for j in range(4 * i4)]
                        base = 4 * i4
                        for r in (3, 2, 1, 0):
                            tiles.append(
                                (slice((base + r) * P, (base + r + 1) * P),
                                 r * P if r else 0, r)
                            )
                    nj = len(tiles)
                    psy = [ppy.tile([P, 512], F32, tag=f"y{dc}",
                                    name=f"psy{dc}")
                           for dc in range(KV // P)]
                    q0 = i4 * 512

                    def scores(idx):
                        ksl, off, _ = tiles[idx]
                        qs = slice(q0 + off, q0 + 512)
                        ps = pp.tile([P, 512], F32, tag="mm",
                                     name=f"psS{idx % 3}")
                        for dc in range(KV // P):
                            nc.tensor.matmul(
                                ps[:, off:512], ckvT[:, dc, ksl],
                                qnT[h][:, dc, qs],
                                start=(dc == 0), stop=False,
                            )
                        nc.tensor.matmul(
                            ps[:, off:512],
                            krT2[h * RH : (h + 1) * RH, ksl],
                            qrT2[h * RH : (h + 1) * RH, qs],
                            start=False, stop=True,
                        )
                        return ps

                    def post(idx, ps):
                        ksl, off, mr = tiles[idx]
                        se = cs.tile([P, 512], BF16, tag="se", bufs=4,
                                     name="se")
                        nc.scalar.activation(se[:, off:512], ps[:, off:512],
                                             AF.Exp)
                        if mr is not None:
                            nc.vector.tensor_mul(
                                se[:, off:512], se[:, off:512],
                                masks[mr][:, off:512],
                            )
                        if idx == 0:
                            nc.vector.tensor_copy(accD[:], se[:])
                        else:
                            nc.vector.tensor_add(
                                accD[:, off:512], accD[:, off:512],
                                se[:, off:512],
                            )
                        first = (idx == 0)
                        last = (idx == nj - 1)
                        for dc in range(KV // P):
                            nc.tensor.matmul(
                                psy[dc][:, off:512],
                                v[:, ksl.start // P, dc * P : (dc + 1) * P],
                                se[:, off:512],
                                start=first, stop=last,
                            )

                    # two-deep software pipeline: scores run two tiles
                    # ahead of the exp-dependent work
                    ring = [scores(0)]
                    if nj > 1:
                        ring.append(scores(1))
                    for idx in range(2, nj):
                        ring.append(scores(idx))
                        post(idx - 2, ring.pop(0))
                    while ring:
                        post(nj - len(ring), ring.pop(0))

                    psden = pden.tile([1, 512], F32, tag="den", name="psden")
                    nc.tensor.matmul(psden[:], ones_col[:], accD[:],
                                     start=True, stop=True)
                    deninv = cs.tile([1, 512], F32, tag="deninv",
                                     name="deninv")
                    nc.vector.reciprocal_approx_fast(out=deninv[:],
                                                     in_=psden[:])
                    nc.gpsimd.partition_broadcast(denb[:], deninv[:])
                    qsl = slice(i4 * 512, (i4 + 1) * 512)
                    for dc in range(KV // P):
                        nc.vector.tensor_mul(
                            yT[:, h * (KV // P) + dc, qsl], psy[dc][:],
                            denb[:],
                        )

        # ============ Phase D: out^T = W_out_c^T @ y^T ======================
        with ExitStack() as dctx:
            dst = dctx.enter_context(tc.tile_pool(name="dst", bufs=3))
            ppd = dctx.enter_context(tc.tile_pool(name="ppd", bufs=1,
                                                  space="PSUM"))
            for mc in range(E // P):
                psD = [ppd.tile([P, 512], F32,
                                tag=f"d{(mc * NT512 + tcc) % 5}",
                                name=f"psD{tcc}")
                       for tcc in range(NT512)]
                for kc in range(DK):
                    for tcc in range(NT512):
                        nc.tensor.matmul(
                            psD[tcc][:],
                            wout_sb[:, kc, mc * P : (mc + 1) * P],
                            yT[:, kc, tcc * 512 : (tcc + 1) * 512],
                            start=(kc == 0), stop=(kc == DK - 1),
                        )
                    if kc == DK - 1:
                        for tcc in range(NT512):
                            ost = dst.tile([P, 512], F32,
                                           tag=f"ost{tcc % 2}",
                                           name="ost")
                            if tcc % 2 == 0:
                                nc.vector.tensor_copy(ost[:], psD[tcc][:])
                            else:
                                nc.scalar.copy(ost[:], psD[tcc][:])
                            (nc.scalar if tcc % 2 else nc.sync).dma_start(
                                outT[mc * P : (mc + 1) * P,
                                     tcc * 512 : (tcc + 1) * 512],
                                ost[:],
                            )

    nc.compile()
    return nc


_NC_CACHE = {}


def _get_nc(T=T_FULL):
    if T not in _NC_CACHE:
        _NC_CACHE[T] = build_kernel(T)
    return _NC_CACHE[T]


def make_in_maps(x, cos, sin, W_qkv, W_qdec, W_out):
    bf = ml_dtypes.bfloat16
    x = np.asarray(x)
    xT = np.ascontiguousarray(x[0].T).astype(bf)           # [E, T]
    W_qkv = np.asarray(W_qkv).astype(np.float32)
    W_qdec = np.asarray(W_qdec).astype(np.float32)
    W_out = np.asarray(W_out).astype(np.float32)
    cos = np.asarray(cos).astype(np.float32)
    sin = np.asarray(sin).astype(np.float32)

    # Wlat columns: cq 1024 | ckv 512 | kr 64 | pad 64  (replicated)
    wlat = np.zeros((E, LATF), np.float32)
    wlat[:, 0:QL] = W_qkv[:, QKH:]
    wlat[:, QL : QL + KV] = W_qkv[:, 0:KV]
    wlat[:, QL + KV : QL + KV + RH] = W_qkv[:, KV : KV + RH]
    wlat = wlat.astype(bf)

    cosT = cos.T.copy()                                     # [64, T]
    ssinT = sin.T.copy()
    ssinT[0 : RH // 2] *= -1.0
    cos2 = np.vstack([cosT, cosT]).astype(bf)               # [128, T]
    ssin2 = np.vstack([ssinT, ssinT]).astype(bf)

    Wq = W_qdec * SCALE
    in_maps = []
    for c in range(NCORES):
        h0, h1 = 2 * c, 2 * c + 1
        # wqd cols: [qr_h0 64 | qr_h1 64 | qn_h0 512 | qn_h1 512]
        wqd_c = np.concatenate(
            [
                Wq[:, h0 * QKH + KV : (h0 + 1) * QKH],
                Wq[:, h1 * QKH + KV : (h1 + 1) * QKH],
                Wq[:, h0 * QKH : h0 * QKH + KV],
                Wq[:, h1 * QKH : h1 * QKH + KV],
            ],
            axis=1,
        ).astype(bf)
        tsl = slice(c * TLOC, (c + 1) * TLOC)
        in_maps.append({
            "xTf": xT,
            "xTl": np.ascontiguousarray(xT[:, tsl]),
            "wlat": wlat,
            "wqd": wqd_c,
            "wout": W_out[c * HPC * KV : (c + 1) * HPC * KV].astype(bf),
            "cos2": cos2,
            "ssin2": ssin2,
            "klcos": np.ascontiguousarray(cos2[0:RH, tsl]),
            "klsin": np.ascontiguousarray(ssin2[0:RH, tsl]),
        })
    return in_maps


def kernel(x, cos, sin, W_qkv, W_qdec, W_out, _trace=False, _tmpdir=None):
    T = np.asarray(x).shape[1]
    nc = _get_nc(T)
    in_maps = make_in_maps(x, cos, sin, W_qkv, W_qdec, W_out)
    res = run_bass_kernel_spmd(
        nc, in_maps, core_ids=list(range(NCORES)),
        trace=_trace, tmpdir=_tmpdir,
    )
    out = np.zeros((E, T), np.float32)
    for r in res.results:
        out += r["outT"]
    kernel.last_results = res
    return np.ascontiguousarray(out.T)[None].astype(np.float32)
